# revision 77
# baseline (speedup 1.0000x reference)
"""Multi-head attention forward on 8 Trainium2 NeuronCores.

Problem: nn.MultiHeadAttention, input [4, 2048, 1024], 16 heads, head_dim 64.

Sharding: core = (batch b, head-group g) with b = core // 2, g = core % 2.
Each core computes attention for 8 heads (4 head-pairs) of one batch and the
corresponding row-parallel slice of the output projection.  Each core emits
TWO partial outputs (pairs 0-2 and pair 3) so most of the output projection
overlaps the last pair's attention; the host sums the four partials per batch
and adds the folded biases.

Pipeline per (pair, 512-token q-chunk), paced by the Scalar engine's exp:
  scores  S^T[kv,q]  two K=64 matmuls row-packed (even head rows 0:64,
                     odd head rows 64:128) into one [128,1024] psum tile
  exp                one [128,1024] activation -> bf16 sbuf (both heads)
  A@V                two M=64 matmuls col-packed (even -> psum rows 0:64,
                     odd -> rows 64:128), accumulated over 16 kv tiles
  denominator        in-place pairwise DVE tree over the 16 exp tiles, then
                     a selector matmul reduces 128 partitions into one psum
                     row per (chunk, head); reciprocals batched per pair
  normalize          mask-matmul broadcasts 1/den to 64 partitions; one DVE
                     multiply writes normalized attnT (bf16)
The K-projection bias is dropped entirely: softmax over kv is invariant to
the per-q constant Q.b_k.  V/O biases are folded on the host.
"""

import ml_dtypes
import numpy as np

import concourse.bass as bass
import concourse.mybir as mybir
import concourse.tile as tile
from concourse import bacc
from concourse.bass_utils import run_bass_kernel_spmd

B = 4
S = 2048
E = 1024
H = 16
D = 64
N_CORES = 8
HPC = 8                  # heads per core
PAIRS = 4
DH = 512                 # per-core qkv slice width
KT = E // 128            # 8 k-tiles over embed dim
SVT = S // 128           # 16 kv tiles
CH = 4                   # q chunks
QC = S // CH             # 512
F32 = mybir.dt.float32
F32R = mybir.dt.float32r
BF16 = mybir.dt.bfloat16

_CACHE = {}

# test.py may set this to capture a profile; the graded path leaves it off.
TRACE = False
LAST_RESULTS = None


def _build_program():
    nc = bacc.Bacc("TRN2", target_bir_lowering=False, debug=False)

    xT_d = nc.dram_tensor("xT", [E, S], F32, kind="ExternalInput")
    wqT_d = nc.dram_tensor("wqT", [E, DH], F32, kind="ExternalInput")
    wkT_d = nc.dram_tensor("wkT", [E, DH], F32, kind="ExternalInput")
    wvT_d = nc.dram_tensor("wvT", [E, DH], F32, kind="ExternalInput")
    wo_d = nc.dram_tensor("wo_bf", [DH, E], BF16, kind="ExternalInput")
    bq_d = nc.dram_tensor("bq", [128, PAIRS], F32, kind="ExternalInput")
    sel_d = nc.dram_tensor("sel", [128, 8, 8], BF16, kind="ExternalInput")
    selb_d = nc.dram_tensor("selb", [128, 2, 2], BF16, kind="ExternalInput")
    zer_d = nc.dram_tensor("zer", [128, QC], BF16, kind="ExternalInput")
    msk_d = nc.dram_tensor("msk", [128, 8, 128], BF16, kind="ExternalInput")
    ya_d = nc.dram_tensor("ya", [S, E], BF16, kind="ExternalOutput")
    yb_d = nc.dram_tensor("yb", [S, E], BF16, kind="ExternalOutput")

    EXPF = mybir.ActivationFunctionType.Exp
    ADD = mybir.AluOpType.add
    MUL = mybir.AluOpType.mult

    with tile.TileContext(nc) as tc:
        pp = tc.alloc_tile_pool(name="persist", bufs=1)
        xt = [pp.tile([128, S], F32R, name=f"xt{k}") for k in range(KT)]
        attnT = [pp.tile([128, S], BF16, name=f"attnT{p}") for p in range(PAIRS)]
        ee = pp.tile([128, SVT, 1024], BF16, name="ee")
        vp = pp.tile([128, SVT, HPC, D], BF16, name="vp")
        dacc = pp.tile([128, 1024], BF16, name="dacc")
        recr = pp.tile([128, QC], BF16, name="recr")
        sel = pp.tile([128, 8, 8], BF16, name="sel")
        selb = pp.tile([128, 2, 2], BF16, name="selb")
        msk = pp.tile([128, 8, 128], BF16, name="msk")
        bqt = pp.tile([128, PAIRS], F32, name="bqt")

        nc.sync.dma_start(sel[:], sel_d.ap())
        nc.sync.dma_start(selb[:], selb_d.ap())
        nc.sync.dma_start(msk[:], msk_d.ap())
        nc.sync.dma_start(bqt[:], bq_d.ap())
        nc.sync.dma_start(recr[:], zer_d.ap())

        def dma_x_chunk(c):
            for k in range(KT):
                nc.sync.dma_start(
                    xt[k][:, c * QC:(c + 1) * QC],
                    xT_d.ap()[k * 128:(k + 1) * 128, c * QC:(c + 1) * QC]
                    .bitcast(F32R))

        qkp = tc.alloc_tile_pool(name="qkp", bufs=2)
        wqkp = tc.alloc_tile_pool(name="wqkp", bufs=8)
        avp = tc.alloc_tile_pool(name="avp", bufs=4)
        bcp = tc.alloc_tile_pool(name="bcp", bufs=1)
        ysp = tc.alloc_tile_pool(name="ysp", bufs=4)
        wop = tc.alloc_tile_pool(name="wop", bufs=1)
        wvp = tc.alloc_tile_pool(name="wvp", bufs=8)   # released after V-proj

        ps_sc = tc.alloc_tile_pool(name="ps_sc", bufs=2, space="PSUM")
        ps_q = tc.alloc_tile_pool(name="ps_q", bufs=1, space="PSUM")
        ps_v = tc.alloc_tile_pool(name="ps_v", bufs=2, space="PSUM")

        # ---- emission helpers -------------------------------------------
        def emit_pair_weights(p):
            wq_t, wk_t = [], []
            for k in range(KT):
                wq = wqkp.tile([128, 128], F32R, name="wq", tag="wq")
                nc.sync.dma_start(
                    wq[:], wqT_d.ap()[k * 128:(k + 1) * 128,
                                      p * 128:(p + 1) * 128].bitcast(F32R))
                wq_t.append(wq)
            for k in range(KT):
                wk = wqkp.tile([128, 128], F32R, name="wk", tag="wk")
                nc.sync.dma_start(
                    wk[:], wkT_d.ap()[k * 128:(k + 1) * 128,
                                      p * 128:(p + 1) * 128].bitcast(F32R))
                wk_t.append(wk)
            return wq_t, wk_t

        def emit_qkproj_chunk(p, c, qt, kt, wq_t, wk_t):
            pq = ps_q.tile([128, QC], F32, name="pq", tag="pq")
            for k in range(KT):
                nc.tensor.matmul(pq[:], wq_t[k][:], xt[k][:, c * QC:(c + 1) * QC],
                                 start=(k == 0), stop=(k == KT - 1))
            nc.vector.tensor_scalar_add(
                qt[:, c * QC:(c + 1) * QC], pq[:], bqt[:, p:p + 1])
            pk = ps_q.tile([128, QC], F32, name="pk", tag="pq")
            for k in range(KT):
                nc.tensor.matmul(pk[:], wk_t[k][:], xt[k][:, c * QC:(c + 1) * QC],
                                 start=(k == 0), stop=(k == KT - 1))
            nc.vector.tensor_copy(kt[:, c * QC:(c + 1) * QC], pk[:])

        def emit_vproj_tile(t, wv_t):
            pv = ps_v.tile([128, DH], F32, name="pv", tag="pv")
            for k in range(KT):
                nc.tensor.matmul(pv[:], xt[k][:, t * 128:(t + 1) * 128], wv_t[k][:],
                                 start=(k == 0), stop=(k == KT - 1))
            nc.vector.tensor_copy(
                vp[:, t, :, :], pv[:].rearrange("p (h d) -> p h d", h=HPC))

        def emit_scores_tile(c, t, qt, kt):
            ps = ps_sc.tile([128, 1024], F32, name="pse", tag="pse")
            nc.tensor.matmul(ps[:, 0:QC],
                             kt[0:64, t * 128:(t + 1) * 128],
                             qt[0:64, c * QC:(c + 1) * QC],
                             start=True, stop=True, tile_position=(0, 0))
            nc.tensor.matmul(ps[:, QC:1024],
                             kt[64:128, t * 128:(t + 1) * 128],
                             qt[64:128, c * QC:(c + 1) * QC],
                             start=True, stop=True, tile_position=(64, 0))
            nc.scalar.activation(ee[:, t, :], ps[:], EXPF)

        # denominator ladder: in-place pairwise adds over the 16 exp tiles,
        # each emitted as soon as its inputs' A@V reads completed, so exp
        # tiles of the NEXT chunk are freed tile-by-tile instead of in one
        # 4.4us block at the chunk boundary.
        LADDER = {1: [(1, 0)], 3: [(3, 2), (3, 1)], 5: [(5, 4)],
                  7: [(7, 6), (7, 5), (7, 3)], 9: [(9, 8)],
                  11: [(11, 10), (11, 9)], 13: [(13, 12)],
                  15: [(15, 14), (15, 13), (15, 11)]}

        def emit_chunk(p, c, qt, kt, fillers, scores=True):
            """Software-pipelined chunk: scores-t runs two tiles ahead of
            AV-(t-2); ladder adds and filler pieces ride the AV slots with
            an even per-slot quota."""
            fq = list(fillers)
            nf = len(fq)
            quota = [(nf * (t + 1)) // SVT - (nf * t) // SVT
                     for t in range(SVT)]
            pave = ps_av.tile([128, QC], F32, name="pave", tag="pave")
            pavo = ps_av.tile([128, QC], F32, name="pavo", tag="pavo")

            def av_one(t):
                # separate banks per head stream: hardware start=True clears
                # the whole bank's has_written bits, so the two interleaved
                # accumulation groups cannot share one
                nc.tensor.matmul(pave[0:64, :], vp[:, t, 2 * p, :],
                                 ee[:, t, 0:QC],
                                 start=(t == 0), stop=(t == SVT - 1),
                                 tile_position=(0, 0))
                nc.tensor.matmul(pavo[64:128, :], vp[:, t, 2 * p + 1, :],
                                 ee[:, t, QC:1024],
                                 start=(t == 0), stop=(t == SVT - 1),
                                 tile_position=(0, 64), skip_group_check=True)
                for d, s_ in LADDER.get(t, []):
                    nc.vector.tensor_tensor(
                        ee[:, d, :], ee[:, d, :], ee[:, s_, :], ADD)
                if t == SVT - 1:
                    nc.vector.tensor_tensor(
                        dacc[:], ee[:, 15, :], ee[:, 7, :], ADD)
                for _ in range(quota[t]):
                    if fq:
                        fq.pop(0)()

            if scores:
                for t in range(SVT):
                    emit_scores_tile(c, t, qt, kt)
                    if t >= 2:
                        av_one(t - 2)
                av_one(SVT - 2)
                av_one(SVT - 1)
            else:
                for t in range(SVT):
                    av_one(t)
            while fq:
                fq.pop(0)()
            av = avp.tile([128, QC], BF16, name="av", tag="av")
            nc.vector.tensor_copy(av[0:64, :], pave[0:64, :])
            nc.vector.tensor_copy(av[64:128, :], pavo[64:128, :])
            return av

        def emit_sel(p, c, pden):
            # reduce dacc partitions into one psum row; pair 3 closes its
            # group per chunk at rows 32c/32c+1 (32-aligned so the per-chunk
            # reciprocal has a legal partition base)
            for h in range(2):
                if p < 3:
                    nc.tensor.matmul(
                        pden[0:8, :], sel[:, 2 * c + h, :],
                        dacc[:, h * QC:(h + 1) * QC],
                        start=(c == 0 and h == 0), stop=(c == 3 and h == 1),
                        tile_position=(0, 0))
                else:
                    nc.tensor.matmul(
                        pden[32 * c:32 * c + 2, :], selb[:, h, :],
                        dacc[:, h * QC:(h + 1) * QC],
                        start=(h == 0), stop=(h == 1),
                        tile_position=(0, 32 * c), skip_group_check=True)

        def emit_norm_chunk(p, c, av):
            # one M=128 matmul broadcasts 1/den of both heads: mask columns
            # 0:64 select the even head's recr row, 64:128 the odd head's.
            # Rides the pq tag chain (all short-lived tiles), not pave/pavo
            # whose current-chunk tiles are allocated before these filler
            # pieces run (that ordering would deadlock).
            pbc = ps_q.tile([128, QC], F32, name="pbc", tag="pq")
            nc.tensor.matmul(pbc[:], msk[:, (4 if p == 3 else 0) + c, :],
                             recr[:], start=True, stop=True,
                             tile_position=(0, 0))
            bces = bcp.tile([128, QC], F32, name="bces", tag="bces")
            nc.vector.tensor_copy(bces[:], pbc[:])
            nc.vector.tensor_tensor(
                attnT[p][:, c * QC:(c + 1) * QC], av[:], bces[:], MUL)

        y_alt = [0]

        def emit_y_group(tt, nch, plist, y_d):
            # ping-pong between the projection bank and the (short-lived
            # during pair 3) denominator bank so matmul and psum-drain copy
            # pipeline instead of serializing on one bank
            y_alt[0] ^= 1
            pool, tag = ((ps_q, "pq") if y_alt[0] else (ps_den, "pden"))
            py = pool.tile([128, QC], F32, name="py", tag=tag)
            for i, p in enumerate(plist):
                nc.tensor.matmul(py[:], attnT[p][:, tt * 128:(tt + 1) * 128],
                                 wo[p][:, nch * QC:(nch + 1) * QC],
                                 start=(i == 0), stop=(i == len(plist) - 1))
            ys = ysp.tile([128, QC], BF16, name="ys", tag="ys")
            nc.vector.tensor_copy(ys[:], py[:])
            nc.sync.dma_start(
                y_d.ap()[tt * 128:(tt + 1) * 128, nch * QC:(nch + 1) * QC], ys[:])

        def _ensure_pair(P):
            if P not in pair_w:
                pair_w[P] = emit_pair_weights(P)
                qts[P] = (qkp.tile([128, S], F32R, name="qt", tag="qt"),
                          qkp.tile([128, S], F32R, name="kt", tag="kt"))

        def proj_pieces(P, cchunk, which):
            """One projection sub-chunk (Q or K) as 9 single-matmul filler
            pieces so it never starves the scalar engine."""
            _ensure_pair(P)
            qt_, kt_ = qts[P]
            w_t = pair_w[P][0 if which == "q" else 1]
            sl = slice(cchunk * QC, (cchunk + 1) * QC)
            st = {}

            def mm(k):
                def f():
                    if k == 0:
                        st["ps"] = ps_q.tile([128, QC], F32, name="pqk",
                                             tag="pq")
                    nc.tensor.matmul(st["ps"][:], w_t[k][:], xt[k][:, sl],
                                     start=(k == 0), stop=(k == KT - 1))
                return f

            def out():
                if which == "q":
                    nc.vector.tensor_scalar_add(qt_[:, sl], st["ps"][:],
                                                bqt[:, P:P + 1])
                else:
                    nc.vector.tensor_copy(kt_[:, sl], st["ps"][:])

            return [mm(k) for k in range(KT)] + [out]

        def y_piece(tt, nch, plist, y_d):
            return lambda: emit_y_group(tt, nch, plist, y_d)

        # ---- main emission ----------------------------------------------
        # DMA order matters for the ramp: pair-0 weights + the first token
        # chunk of x must land before the remaining 6 MB of x
        pair_w = {0: emit_pair_weights(0)}
        dma_x_chunk(0)
        wv_t = []
        for k in range(KT):
            wv = wvp.tile([128, DH], F32R, name="wv", tag="wv")
            nc.sync.dma_start(
                wv[:], wvT_d.ap()[k * 128:(k + 1) * 128, :].bitcast(F32R))
            wv_t.append(wv)
        for cx in range(1, CH):
            dma_x_chunk(cx)

        qts = {0: (qkp.tile([128, S], F32R, name="qt", tag="qt"),
                   qkp.tile([128, S], F32R, name="kt", tag="kt"))}

        wo = [wop.tile([128, E], BF16, name="wo", tag=f"wo{p}")
              for p in range(PAIRS)]

        # psum pools for the attention pipeline appear once V-proj's are gone;
        # allocate them after emitting chunk (0,0), whose AVs are emitted
        # inside emit_chunk -> so V-proj must NOT be inside emit_chunk's AV
        # path.  V-proj tiles ride the scores slots of chunk (0,0).
        ya_list = [y_piece(tt, nch, [0, 1, 2], ya_d)
                   for tt in range(SVT) for nch in range(2)]
        yb_list = {cv: [y_piece(4 * cv + i, n, [3], yb_d)
                        for i in range(4) for n in range(2)]
                   for cv in range(CH)}

        av_tiles = {}
        pdens = {}

        def sel_piece(pv, cv):
            key = (pv, cv) if pv == 3 else pv
            if key not in pdens:
                pdens[key] = ps_den.tile([128, QC], F32, name="pden",
                                         tag="pden")
            pd = pdens[key]
            return lambda: emit_sel(pv, cv, pd)

        def recip_pieces(pv, cv):
            # four column-slices instead of one 4.3us instruction: the DVE
            # stream is in-order, and one long reciprocal delays the ladder
            # adds that free exp tiles for the next chunk
            def mk(q):
                sl = slice(q * 128, (q + 1) * 128)

                def f():
                    with nc.allow_low_precision("f32r for PE broadcast"):
                        if pv == 3:
                            nc.vector.reciprocal(
                                recr[32 * cv:32 * cv + 2, sl],
                                pdens[(3, cv)][32 * cv:32 * cv + 2, sl])
                        else:
                            nc.vector.reciprocal(recr[0:8, sl],
                                                 pdens[pv][0:8, sl])
                return f
            return [mk(q) for q in range(4)]

        def norm_piece(pv, cc):
            return lambda: emit_norm_chunk(pv, cc, av_tiles[(pv, cc)])

        carry = []          # norm pieces deferred to the following block
        for bi in range(17):
            p, c = divmod(bi, 4) if bi < 16 else (3, 3)
            pv_, cv_ = divmod(bi - 1, 4)

            # deferred selector + normalization pieces for the previous chunk
            pre = carry
            carry = []
            if bi >= 1:
                pre = pre + [sel_piece(pv_, cv_)]
                if pv_ == 3:
                    pre += recip_pieces(3, cv_) + [norm_piece(3, cv_)]
                elif cv_ == 3:
                    # pair pv_ finished: normalize c0/c1 now, c2/c3 next block
                    pre += recip_pieces(pv_, 3) + [norm_piece(pv_, 0),
                                                   norm_piece(pv_, 1)]
                    carry = [norm_piece(pv_, 2), norm_piece(pv_, 3)]

            if bi == 16:
                for f in pre:
                    f()
                for f in yb_list[3]:
                    f()
                break

            # filler pieces for this block.  K of pair P must fully precede
            # block (P,0) because scores contract over ALL kv chunks; Q chunk
            # c' only needs to precede block (P,c').
            if bi == 0:
                fillers = []
            elif bi <= 12:
                P = 1 + (bi - 1) // 4
                r = (bi - 1) % 4
                fillers = {
                    0: lambda: proj_pieces(P, 0, "k") + proj_pieces(P, 1, "k"),
                    1: lambda: proj_pieces(P, 2, "k") + proj_pieces(P, 3, "k"),
                    2: lambda: proj_pieces(P, 0, "q") + proj_pieces(P, 1, "q"),
                    3: lambda: proj_pieces(P, 2, "q") + proj_pieces(P, 3, "q"),
                }[r]()
            elif bi == 13:
                fillers = ya_list[0:10] + yb_list[0]
            elif bi == 14:
                fillers = ya_list[10:21] + yb_list[1]
            else:
                fillers = ya_list[21:32] + yb_list[2]
            fillers = pre + fillers

            if c == 0 and p >= 1:
                qts.pop(p - 1, None)

            if bi == 0:
                # ramp: each QK-0 chunk unlocks its scores quarter right away
                # (kv tile t only reads K chunk t//4); V-proj rides along
                for sub in range(CH):
                    emit_qkproj_chunk(0, sub, *qts[0], *pair_w[0])
                    for t in range(4 * sub, 4 * sub + 4):
                        emit_vproj_tile(t, wv_t)
                        emit_scores_tile(0, t, *qts[0])
                ps_v.release()
                wvp.release()
                for pw in range(PAIRS):
                    nc.sync.dma_start(wo[pw][:],
                                      wo_d.ap()[pw * 128:(pw + 1) * 128, :])
                ps_av = tc.alloc_tile_pool(name="ps_av", bufs=1, space="PSUM")
                ps_den = tc.alloc_tile_pool(name="ps_den", bufs=1,
                                            space="PSUM")
                av_tiles[(0, 0)] = emit_chunk(0, 0, *qts[0], [], scores=False)
            else:
                av_tiles[(p, c)] = emit_chunk(p, c, *qts[p], fillers)

        for pool in (ps_den, ps_av, ps_q, ps_sc,
                     wop, ysp, bcp, avp, wqkp, qkp, pp):
            pool.release()

    nc.compile()
    return nc


def kernel(input_tensor, wq, bq, wk, bk_, wv, bv, wo, bo):
    global LAST_RESULTS
    if "nc" not in _CACHE:
        _CACHE["nc"] = _build_program()
    nc = _CACHE["nc"]

    x = np.asarray(input_tensor, dtype=np.float32)
    scale = np.float32(1.0 / np.sqrt(np.float32(D)))

    wqT = np.ascontiguousarray(np.asarray(wq, np.float32).T * scale)
    wkT = np.ascontiguousarray(np.asarray(wk, np.float32).T)
    wvT = np.ascontiguousarray(np.asarray(wv, np.float32).T)
    woT = np.ascontiguousarray(np.asarray(wo, np.float32).T)
    bqs = np.asarray(bq, np.float32) * scale

    # selector for the denominator partition-reduce (one-hot column) and
    # mask for the 1/den broadcast (one-hot row).  Slots 0-7: pairs 0-2 use
    # row 2c+h; slots 8-15: pair 3 uses row 32c+h (32-aligned so each
    # chunk's reciprocal has a legal partition base).
    sel3 = np.zeros((128, 8, 8), ml_dtypes.bfloat16)
    selb = np.zeros((128, 2, 2), ml_dtypes.bfloat16)
    selb[:, 0, 0] = 1.0
    selb[:, 1, 1] = 1.0
    msk8 = np.zeros((128, 8, 128), ml_dtypes.bfloat16)
    for c in range(4):
        for hh in range(2):
            sel3[:, 2 * c + hh, 2 * c + hh] = 1.0
            # pairs 0-2: den rows 2c+h; pair 3: den rows 32c+h
            msk8[2 * c + hh, c, 64 * hh:64 * (hh + 1)] = 1.0
            msk8[32 * c + hh, 4 + c, 64 * hh:64 * (hh + 1)] = 1.0

    xTs = [np.ascontiguousarray(x[b].T) for b in range(B)]

    in_maps = []
    for core in range(N_CORES):
        b, g = divmod(core, 2)
        hs = slice(g * DH, (g + 1) * DH)
        in_maps.append({
            "xT": xTs[b],
            "wqT": np.ascontiguousarray(wqT[:, hs]),
            "wkT": np.ascontiguousarray(wkT[:, hs]),
            "wvT": np.ascontiguousarray(wvT[:, hs]),
            "wo_bf": np.ascontiguousarray(woT[hs, :]).astype(ml_dtypes.bfloat16),
            "bq": np.ascontiguousarray(bqs[hs].reshape(PAIRS, 128).T),
            "sel": sel3,
            "selb": selb,
            "msk": msk8,
            "zer": np.zeros((128, QC), ml_dtypes.bfloat16),
        })

    res = run_bass_kernel_spmd(nc, in_maps, core_ids=list(range(N_CORES)),
                               trace=TRACE)
    LAST_RESULTS = res

    # unshard: sum the four partials per batch, add folded biases
    bias_full = (np.asarray(bo, np.float32)
                 + np.asarray(bv, np.float32) @ woT).astype(np.float32)
    y = np.empty((B, S, E), np.float32)
    for b in range(B):
        acc = np.broadcast_to(bias_full[None, :], (S, E)).astype(np.float32)
        for core in (2 * b, 2 * b + 1):
            for nm in ("ya", "yb"):
                acc = acc + np.asarray(res.results[core][nm], dtype=np.float32)
        y[b] = acc
    return y
